# revision 18
# baseline (speedup 1.0000x reference)
# DeepSpeech cell on 8 trn2 NeuronCores.
#
# reference: h = x @ kernel + bias  (B,T,D)@(D,S);  out_t = relu(h_t + out_{t-1} @ Wr)
# B=64 T=512 D=1024 S=2048.
#
# Strategy: tensor-parallel split of the state dimension S across the 8 cores
# (each core owns a 256-column slice of `kernel` and `recurrent_kernel`), with a
# per-step AllGather of the new state slice.  All matmuls are "form B"
# (out.T = W.T @ state.T): the stationary operand is the weight tile in its
# natural [K,M] layout and the state stays in [S, B] layout forever, so no
# transposes appear anywhere on the critical path.  The host pre-transposes x
# to [D, B*T] and pre-casts everything to bf16 (error does not compound through
# the recurrence: the fresh fp32-accumulated h_t dominates each step).
import numpy as np
import ml_dtypes

import concourse.mybir as mybir
import concourse.tile as tile
from concourse import bacc

P = 128
B = 64
T_FULL = 512
D = 1024
S = 2048
NC = 8
S_LOC = S // NC          # 256 state columns per core
MT = S_LOC // P          # 2 output tiles per core
KT = S // P              # 16 contraction tiles over the state
DT = D // P              # 8 contraction tiles over the input dim
BF16 = mybir.dt.bfloat16
F32 = mybir.dt.float32


def build_nc(T: int, hch: int = 64):
    """Build the SPMD Bass program for a T-step recurrence."""
    hch = min(hch, T)
    assert T % hch == 0
    nc = bacc.Bacc("TRN2", target_bir_lowering=False, debug=False, num_devices=NC)

    BL = B // NC  # 8 batches per core for the projection phase
    # xt is sharded by batch-block (tunnel bandwidth is precious): core c gets
    # x.T for batches [8c, 8c+8).  It computes h for ALL of S over those
    # batches, then one AllToAll redistributes h so each core holds its
    # 256-column S-slice for ALL batches.
    xt_ext = nc.dram_tensor("xt", [D, BL * T], BF16, kind="ExternalInput")
    wr_ext = nc.dram_tensor("wr", [S, S_LOC], BF16, kind="ExternalInput")
    # kernel matrix arrives sharded by D-rows (tunnel traffic); an on-device
    # AllGather reconstructs the full [D, S] copy each core needs.
    kern_ext = nc.dram_tensor("kern", [D // NC, S], BF16, kind="ExternalInput")
    kern_full = nc.dram_tensor("kern_full", [D, S], BF16, addr_space="Shared")
    bias_ext = nc.dram_tensor("bias", [P, KT], F32, kind="ExternalInput")
    out_ext = nc.dram_tensor("out", [T, MT, P, B], BF16, kind="ExternalOutput")
    # pre-A2A projection: h_pre[j, s_local, bl*T + t] = h for s-slice j of my batches
    h_pre = nc.dram_tensor("h_pre", [NC, S_LOC, BL * T], BF16)
    # post-A2A: h_store[i, s_local, bl*T + t] = my s-slice for batch-block i
    h_store = nc.dram_tensor("h_store", [NC, S_LOC, BL * T], BF16)

    with tile.TileContext(nc) as tc:
        with (
            tc.tile_pool(name="const", bufs=1) as const_pool,
            tc.tile_pool(name="xt", bufs=2) as xt_pool,
            tc.tile_pool(name="hsb", bufs=2) as hsb_pool,
            tc.tile_pool(name="hbuf", bufs=2) as hbuf_pool,
            tc.tile_pool(name="state", bufs=2) as state_pool,
            tc.tile_pool(name="snew", bufs=2) as snew_pool,
            tc.tile_pool(name="tmp", bufs=2) as tmp_pool,
            tc.tile_pool(name="psum", bufs=2, space="PSUM") as psum_pool,
            tc.tile_pool(name="psum_h", bufs=2, space="PSUM") as psumh_pool,
            tc.tile_pool(name="dram_in", bufs=2, space="DRAM") as dramin_pool,
            tc.tile_pool(name="dram_gather", bufs=3, space="DRAM") as gather_pool,
        ):
            # ---- resident weights ----
            wr_sb = const_pool.tile([P, KT, MT, P], BF16, tag="wr")
            nc.sync.dma_start(
                out=wr_sb[:, :, :, :],
                in_=wr_ext[:, :].rearrange("(k p) (m q) -> p k m q", p=P, q=P),
            )
            kern_bounce = dramin_pool.tile([D // NC, S], BF16, tag="kbounce")
            nc.sync.dma_start(out=kern_bounce[:, :], in_=kern_ext[:, :])
            nc.gpsimd.collective_compute(
                "AllGather",
                mybir.AluOpType.bypass,
                replica_groups=[list(range(NC))],
                ins=[kern_bounce[:, :]],
                outs=[kern_full[:, :]],
            )
            kern_sb = const_pool.tile([P, DT, KT, P], BF16, tag="kern")
            nc.sync.dma_start(
                out=kern_sb[:, :, :, :],
                in_=kern_full[:, :].rearrange("(d p) (m q) -> p d m q", p=P, q=P),
            )
            bias_sb = const_pool.tile([P, KT], F32, tag="bias")
            nc.sync.dma_start(out=bias_sb[:, :], in_=bias_ext[:, :])

            # ---- projection: h.T[s, bt] = kernel.T @ x.T (+ bias), my batches ----
            BL = B // NC
            for bl in range(BL):
                xt_sb = xt_pool.tile([P, DT, T], BF16, tag="xt")
                nc.sync.dma_start(
                    out=xt_sb[:, :, :],
                    in_=xt_ext[:, bl * T : (bl + 1) * T].rearrange(
                        "(d p) t -> p d t", p=P
                    ),
                )
                for st in range(KT):
                    ph = psumh_pool.tile([P, T], F32, tag="ph")
                    for d in range(DT):
                        nc.tensor.matmul(
                            ph[:, :],
                            kern_sb[:, d, st, :],
                            xt_sb[:, d, :],
                            start=(d == 0),
                            stop=(d == DT - 1),
                        )
                    h_sb = hsb_pool.tile([P, T], BF16, tag="hsb")
                    nc.scalar.activation(
                        h_sb[:, :],
                        ph[:, :],
                        mybir.ActivationFunctionType.Identity,
                        bias=bias_sb[:, st : st + 1],
                    )
                    nc.sync.dma_start(
                        out=h_pre[
                            st // MT,
                            (st % MT) * P : (st % MT + 1) * P,
                            bl * T : (bl + 1) * T,
                        ],
                        in_=h_sb[:, :],
                    )
            # redistribute: each core ends with its s-slice for ALL batches
            nc.gpsimd.collective_compute(
                "AllToAll",
                mybir.AluOpType.bypass,
                replica_groups=[list(range(NC))],
                ins=[h_pre[:, :, :]],
                outs=[h_store[:, :, :]],
            )

            # ---- recurrence ----
            state_sb = None
            h_buf = None
            for t in range(T):
                chunk, tl = divmod(t, hch)
                if tl == 0:
                    h_buf = hbuf_pool.tile([P, MT, B, hch], BF16, tag="hbuf")
                    for m in range(MT):
                        for i in range(NC):
                            nc.sync.dma_start(
                                out=h_buf[:, m, i * BL : (i + 1) * BL, :],
                                in_=h_store[i, m * P : (m + 1) * P, :]
                                .rearrange("p (bl t) -> p bl t", bl=BL)[
                                    :, :, chunk * hch : chunk * hch + hch
                                ],
                            )

                snew = snew_pool.tile([P, MT, B], BF16, tag="snew")
                if t == 0:
                    for m in range(MT):
                        nc.scalar.activation(
                            snew[:, m, :],
                            h_buf[:, m, :, 0],
                            mybir.ActivationFunctionType.Relu,
                        )
                else:
                    psums = [
                        psum_pool.tile([P, B], F32, tag=f"ps{m}", name=f"ps{m}")
                        for m in range(MT)
                    ]
                    for k in range(KT):
                        for m in range(MT):
                            nc.tensor.matmul(
                                psums[m][:, :],
                                wr_sb[:, k, m, :],
                                state_sb[:, k, :],
                                start=(k == 0),
                                stop=(k == KT - 1),
                            )
                    for m in range(MT):
                        tmp = tmp_pool.tile([P, B], F32, tag=f"tmp{m}")
                        nc.vector.tensor_tensor(
                            tmp[:, :],
                            psums[m][:, :],
                            h_buf[:, m, :, tl],
                            mybir.AluOpType.add,
                        )
                        nc.scalar.activation(
                            snew[:, m, :],
                            tmp[:, :],
                            mybir.ActivationFunctionType.Relu,
                        )

                # output slice for this step (off critical path)
                nc.sync.dma_start(
                    out=out_ext[t, :, :, :].rearrange("m p b -> p m b"),
                    in_=snew[:, :, :],
                )

                if t < T - 1:
                    # broadcast my new slice to everyone: bounce -> AllGather
                    bounce = dramin_pool.tile([MT, P, B], BF16, tag="bounce")
                    nc.sync.dma_start(
                        out=bounce[:, :, :].rearrange("m p b -> p m b"),
                        in_=snew[:, :, :],
                    )
                    gathered = gather_pool.tile(
                        [KT, P, B], BF16, tag="gather", addr_space="Shared"
                    )
                    nc.gpsimd.collective_compute(
                        "AllGather",
                        mybir.AluOpType.bypass,
                        replica_groups=[list(range(NC))],
                        ins=[bounce[:, :, :]],
                        outs=[gathered[:, :, :]],
                    )
                    state_sb = state_pool.tile([P, KT, B], BF16, tag="state")
                    for j in range(4):
                        nc.sync.dma_start(
                            out=state_sb[:, 4 * j : 4 * j + 4, :],
                            in_=gathered[:, :, :].rearrange("k p b -> p k b")[
                                :, 4 * j : 4 * j + 4, :
                            ],
                        )
    nc.compile()
    return nc


def _prep_inputs(x, kern, wr, bias, T):
    bf = ml_dtypes.bfloat16
    BL = B // NC
    DL = D // NC
    bias_arr = np.ascontiguousarray(bias.reshape(KT, P).T.astype(np.float32))
    in_maps = []
    for c in range(NC):
        sl = slice(c * S_LOC, (c + 1) * S_LOC)
        xt_c = np.ascontiguousarray(
            x[c * BL : (c + 1) * BL].reshape(BL * T, D).T.astype(bf)
        )  # [D, BL*T], col = bl*T + t
        in_maps.append(
            {
                "xt": xt_c,
                "wr": np.ascontiguousarray(wr[:, sl].astype(bf)),
                "kern": np.ascontiguousarray(kern[c * DL : (c + 1) * DL].astype(bf)),
                "bias": bias_arr,
            }
        )
    return in_maps


def _assemble(results, T):
    # per-core out: [T, MT, P, B] bf16, s_local = m*P + p
    slices = [
        np.asarray(r["out"]).reshape(T, S_LOC, B) for r in results
    ]
    full = np.concatenate(slices, axis=1)  # [T, S, B]
    return np.ascontiguousarray(full.transpose(2, 0, 1).astype(np.float32))


class _Executor:
    """Compile once; re-execute cheaply with device-resident buffers.

    Mirrors concourse.bass2jax.run_bass_via_pjrt, but keeps the jitted
    shard_map callable so repeated executions skip recompilation, and lets the
    caller chain one call's outputs in as the next call's donated scratch
    buffers (my kernel writes every output element, so initial contents don't
    matter) — steady-state calls transfer nothing to the device.
    """

    def __init__(self, nc, n_cores=NC):
        import jax
        from jax.experimental.shard_map import shard_map
        from jax.sharding import Mesh, NamedSharding, PartitionSpec

        from concourse import bass2jax

        bass2jax.install_neuronx_cc_hook()
        self.jax = jax
        assert nc.dbg_addr is None

        partition_name = (
            nc.partition_id_tensor.name if nc.partition_id_tensor else None
        )
        in_names, out_names, out_avals, zero_outs = [], [], [], []
        for alloc in nc.m.functions[0].allocations:
            if not isinstance(alloc, mybir.MemoryLocationSet):
                continue
            name = alloc.memorylocations[0].name
            if alloc.kind == "ExternalInput":
                if name != partition_name:
                    in_names.append(name)
            elif alloc.kind == "ExternalOutput":
                shape = tuple(alloc.tensor_shape)
                dtype = mybir.dt.np(alloc.dtype)
                out_names.append(name)
                out_avals.append(jax.core.ShapedArray(shape, dtype))
                zero_outs.append(np.zeros(shape, dtype))
        self.in_names = list(in_names)
        self.out_names = out_names
        self.zero_outs = zero_outs
        n_params = len(in_names)
        n_outs = len(out_avals)
        all_names = in_names + out_names + (
            [partition_name] if partition_name else []
        )
        donate = tuple(range(n_params, n_params + n_outs))

        def _body(*args):
            operands = list(args)
            if partition_name is not None:
                operands.append(bass2jax.partition_id_tensor())
            return tuple(
                bass2jax._bass_exec_p.bind(
                    *operands,
                    out_avals=tuple(out_avals),
                    in_names=tuple(all_names),
                    out_names=tuple(out_names),
                    lowering_input_output_aliases=(),
                    sim_require_finite=True,
                    sim_require_nnan=True,
                    nc=nc,
                )
            )

        devices = jax.devices()[:n_cores]
        assert len(devices) == n_cores
        self.mesh = Mesh(np.asarray(devices), ("core",))
        self.spec = PartitionSpec("core")
        self.sharding = NamedSharding(self.mesh, self.spec)
        in_specs = (self.spec,) * (n_params + n_outs)
        out_specs = (self.spec,) * n_outs
        self.sharded = jax.jit(
            shard_map(
                _body,
                mesh=self.mesh,
                in_specs=in_specs,
                out_specs=out_specs,
                check_rep=False,
            ),
            donate_argnums=donate,
            keep_unused=True,
        )
        self.n_cores = n_cores
        self._dev_in = None
        self._scratch = None

    def load_inputs(self, in_maps):
        concat = [
            np.concatenate([np.asarray(m[k]) for m in in_maps], axis=0)
            for k in self.in_names
        ]
        self._dev_in = [self.jax.device_put(a, self.sharding) for a in concat]
        for a in self._dev_in:
            a.block_until_ready()
        self._scratch = None

    def execute(self):
        if self._scratch is None:
            # create scratch on-device (the tunnel to the terminal is slow;
            # never ship zero-filled buffers through it)
            jnp = self.jax.numpy
            shapes = [
                ((self.n_cores * z.shape[0], *z.shape[1:]), z.dtype)
                for z in self.zero_outs
            ]
            maker = self.jax.jit(
                lambda: tuple(jnp.zeros(s, d) for s, d in shapes),
                out_shardings=tuple(self.sharding for _ in shapes),
            )
            scratch = list(maker())
            for a in scratch:
                a.block_until_ready()
        else:
            scratch = self._scratch
        outs = self.sharded(*self._dev_in, *scratch)
        for o in outs:
            o.block_until_ready()
        self._scratch = list(outs)
        return outs

    def results(self):
        outs = self._scratch
        per_core = []
        for c in range(self.n_cores):
            d = {}
            for i, name in enumerate(self.out_names):
                shp = self.zero_outs[i].shape
                d[name] = np.asarray(outs[i]).reshape(
                    self.n_cores, *shp
                )[c]
            per_core.append(d)
        return per_core


_EXEC_CACHE = {}


def get_executor(T=T_FULL):
    if T not in _EXEC_CACHE:
        nc = build_nc(T)
        _EXEC_CACHE[T] = _Executor(nc)
    return _EXEC_CACHE[T]


def run(x, kern, wr, bias, T=T_FULL, trace=False):
    ex = get_executor(T)
    ex.load_inputs(_prep_inputs(x, kern, wr, bias, T))
    ex.execute()
    out = _assemble(ex.results(), T)
    return out, ex


def kernel(x, kernel, recurrent_kernel, bias):
    out, _ = run(
        np.asarray(x),
        np.asarray(kernel),
        np.asarray(recurrent_kernel),
        np.asarray(bias),
        T=T_FULL,
    )
    return out
